# revision 20
# baseline (speedup 1.0000x reference)
# Bass/Trainium2 kernel for nn_MENet (scatter_memory).
#
# Strategy: pure data parallel over batch (512 -> 64 per core, 8 cores).
#
# The kernel is HBM-bound: each core must stream 101.7MB of l3/x2 points
# (floor ~280us at ~360GB/s). Everything else is organized to hide under
# that stream:
#   - l3/x2 tiles are DMAd via gpsimd software-DGE with an in-flight
#     f32->bf16 cast; the gpsimd queue carries ONLY these 32 DMAs, issued
#     up-front, so the stream starts at t=0 and never stalls on compute.
#   - DVE does the segmented row maxes in bf16 (2x pipe) plus two tiny
#     reciprocals; all other pointwise work lives on ACT/PE.
#   - memory-addressing softmax: logits are q-hat . m-hat / ||q|| with
#     |logit|<=1, so no max subtraction; exp(l * rinv) is one ACT op per
#     128-row chunk using the per-partition scale operand. ACT functions
#     are phase-batched (square, sqrt, exp, scale, relu) to avoid
#     activation-table reloads.
#   - all matmul weights/activations are bf16 (PE 1 cycle/row vs 4 for
#     f32); psum accumulation stays f32.
#   - head2 consumes max(l3) and max(x2) as separate K-chunks of fc1_2
#     (W(u+v) = Wu + Wv), removing the elementwise adds; its x2-dependent
#     matmuls run per batch-half so only ~32 batches of work trail the
#     last x2 tile.
import os
from contextlib import ExitStack

import numpy as np
import ml_dtypes

import concourse.bacc as bacc
import concourse.bass as bass
import concourse.tile as tile
from concourse import mybir
from concourse.bass_utils import run_bass_kernel_spmd

F32 = mybir.dt.float32
BF16 = mybir.dt.bfloat16
NPBF = ml_dtypes.bfloat16
AF = mybir.ActivationFunctionType
ALU = mybir.AluOpType
AX = mybir.AxisListType

P = 128
NCORES = 8
B = 512
BL = B // NCORES          # 64 batches per core
NM = 32                   # n points per memory block
CM = 64                   # memory channel dim
ROWS = BL * NM            # 2048 rows per core per branch
NCHUNK = ROWS // P        # 16 chunks of 128 rows
NGROUP = ROWS // 512      # 4 groups of 512 rows (16 batches each)
TB = 4                    # batches per streamed tile
NT = BL // TB             # 16 tiles per stream
EPS_BN = 1e-5


# ----------------------------------------------------------------------------
# host-side weight folding + packing (bf16 matmul pack + f32 bias pack)
# ----------------------------------------------------------------------------
class _Pack:
    def __init__(self, dtype):
        self.dtype = dtype
        self.parts = []
        self.off = {}
        self.pos = 0

    def add(self, name, arr):
        arr = np.asarray(arr, np.float32)
        assert arr.ndim == 2 and arr.shape[0] <= P
        buf = np.zeros((P, arr.shape[1]), np.float32)
        buf[: arr.shape[0]] = arr
        self.off[name] = (self.pos, arr.shape[1])
        self.pos += arr.shape[1]
        self.parts.append(buf)

    def finish(self):
        return np.ascontiguousarray(
            np.concatenate(self.parts, axis=1).astype(self.dtype)
        )


def _kpack(w_t):  # [K, M] -> [128, nk, M] flattened to [128, nk*M]
    K, M = w_t.shape
    nk = K // P
    return np.ascontiguousarray(
        np.transpose(w_t.reshape(nk, P, M), (1, 0, 2)).reshape(P, nk * M)
    )


def _perm_pts(npref, npts):
    # device x-vector position npref + j*128 + q  <-  original point 8q + j
    d = np.arange(npts)
    src = npref + 8 * (d % 128) + (d // 128)
    return np.concatenate([np.arange(npref), src])


def _fold_and_pack(f):
    s = lambda g: g / np.sqrt(1.0 + EPS_BN)
    mw = f["memory_w"]                                    # [16, 64]
    mn = mw / np.maximum(np.linalg.norm(mw, axis=1, keepdims=True), 1e-12)

    pb = _Pack(NPBF)       # bf16: matmul operands
    pf = _Pack(np.float32)  # f32: biases, f32 identity

    pb.add("identb", np.eye(P, dtype=np.float32))
    rhs2 = np.zeros((P, 17), np.float32)
    rhs2[0:CM, 0:16] = mn.T                               # logits part
    rhs2[CM:2 * CM, 16] = 1.0                             # sum-of-squares part
    pb.add("rhs2", rhs2)

    for bi, (w1, g1, b1, w2, g2, b2) in enumerate(
        [
            (f["mlp1_w1"], f["mlp1_g1"], f["mlp1_b1"], f["mlp1_w2"], f["mlp1_g2"], f["mlp1_b2"]),
            (f["mlp2_w1"], f["mlp2_g1"], f["mlp2_b1"], f["mlp2_w2"], f["mlp2_g2"], f["mlp2_b2"]),
        ]
    ):
        w1e = (s(g1)[:, None] * w1) @ mw.T                # [M1, 16]
        w2f = s(g2)[:, None] * w2                         # [M2, M1]
        M1, M2 = w2f.shape[1], w2f.shape[0]
        pb.add(f"w1eT_b{bi + 1}", w1e.T)                  # [16, M1]
        pf.add(f"b1_b{bi + 1}", b1.reshape(M1 // P, P).T)
        pb.add(f"w2T_b{bi + 1}", _kpack(w2f.T))           # [128, (M1/128)*M2]
        pf.add(f"b2_b{bi + 1}", b2.reshape(M2 // P, P).T)

    for hi, (w1, b1, g1, bb1, w2, b2, g2, bb2, w3, b3, npref) in enumerate(
        [
            (f["fc1_w"], f["fc1_b"], f["bn1_g"], f["bn1_b"], f["fc2_w"], f["fc2_b"],
             f["bn2_g"], f["bn2_b"], f["fc3_w"], f["fc3_b"], 256),
            (f["fc1_2_w"], f["fc1_2_b"], f["bn1_2_g"], f["bn1_2_b"], f["fc2_2_w"],
             f["fc2_2_b"], f["bn2_2_g"], f["bn2_2_b"], f["fc3_2_w"], f["fc3_2_b"], 512),
        ]
    ):
        s1, s2 = s(g1), s(g2)
        w1f = (s1[:, None] * w1)[:, _perm_pts(npref, 1024)]   # [512, npref+1024]
        b1f = s1 * b1 + bb1
        w2f = s2[:, None] * w2                                # [256, 512]
        b2f = s2 * b2 + bb2
        pb.add(f"fw1_h{hi + 1}", _kpack(w1f.T))               # [128, nk1*512]
        pf.add(f"fb1_h{hi + 1}", b1f.reshape(4, P).T)
        pb.add(f"fw2_h{hi + 1}", _kpack(w2f.T))               # [128, 4*256]
        pf.add(f"fb2_h{hi + 1}", b2f.reshape(2, P).T)
        pb.add(f"fw3_h{hi + 1}", _kpack(w3.T))                # [128, 2*40]
        pf.add(f"fb3_h{hi + 1}", b3.reshape(40, 1))

    pf.add("identf", np.eye(40, dtype=np.float32))

    return pb.finish(), pb.off, pf.finish(), pf.off


# ----------------------------------------------------------------------------
# device program
# ----------------------------------------------------------------------------
def _build(offb, NWB, offf, NWF):
    nc = bacc.Bacc("TRN2", target_bir_lowering=False, debug=False)
    l3d = nc.dram_tensor("l3", [BL, 1024, 128], F32, kind="ExternalInput").ap()
    x2d = nc.dram_tensor("x2", [BL, 1024, 256], F32, kind="ExternalInput").ap()
    mf1d = nc.dram_tensor("mf1", [CM, ROWS], BF16, kind="ExternalInput").ap()
    mf2d = nc.dram_tensor("mf2", [CM, ROWS], BF16, kind="ExternalInput").ap()
    wbd = nc.dram_tensor("wpb", [P, NWB], BF16, kind="ExternalInput").ap()
    wfd = nc.dram_tensor("wpf", [P, NWF], F32, kind="ExternalInput").ap()
    o1d = nc.dram_tensor("out1", [BL, 40], F32, kind="ExternalOutput").ap()
    o2d = nc.dram_tensor("out2", [BL, 40], F32, kind="ExternalOutput").ap()

    # branch-weight prefix of the bf16 pack (needed within ~5us; the big
    # head-weight tail can land later)
    bw_end = offb["fw1_h1"][0]

    with tile.TileContext(nc) as tc, ExitStack() as ctx:
        pp = ctx.enter_context(tc.tile_pool(name="persist", bufs=1))
        wb = pp.tile([P, NWB], BF16, name="wb")
        wf = pp.tile([P, NWF], F32, name="wf")
        S1 = pp.tile([P, ROWS], BF16, name="S1")
        S2 = pp.tile([P, ROWS], BF16, name="S2")
        # scalar (ACT) hardware queue: memory-branch inputs + weights, so the
        # sync queue is free for the l3 stream from t=0
        nc.scalar.dma_start(S1[0:CM, :], mf1d)
        nc.scalar.dma_start(S1[CM:2 * CM, :], mf1d)
        nc.scalar.dma_start(S2[0:CM, :], mf2d)
        nc.scalar.dma_start(S2[CM:2 * CM, :], mf2d)
        nc.scalar.dma_start(wb[:, 0:bw_end], wbd[:, 0:bw_end])
        nc.scalar.dma_start(wf[:], wfd)
        nc.scalar.dma_start(wb[:, bw_end:], wbd[:, bw_end:])

        def W(name):
            o, w = offb[name]
            return wb[:, o: o + w]

        def Wf(name):
            o, w = offf[name]
            return wf[:, o: o + w]

        eps = pp.tile([P, 1], F32, name="eps")
        nc.vector.memset(eps[:], 1e-24)

        xt = pp.tile([P, BL, 8], BF16, name="xt")     # l3 point maxes
        xx = pp.tile([P, BL, 8], BF16, name="xx")     # x2 point maxes
        xm1 = pp.tile([P, 2, BL], BF16, name="xm1")   # branch1 mlp maxes
        xm2 = pp.tile([P, 4, BL], BF16, name="xm2")   # branch2 mlp maxes
        rr = pp.tile([P, 32], F32, name="rr")         # ||q|| per chunk-row
        rinv = pp.tile([P, 32], F32, name="rinv")
        se = pp.tile([P, 32], F32, name="se")
        rs = pp.tile([P, 32], F32, name="rs")
        epk = [pp.tile([P, NCHUNK, 16], BF16, name=f"e{b}") for b in range(2)]
        apk = [pp.tile([P, NCHUNK, 16], BF16, name=f"a{b}") for b in range(2)]

        # squares on partitions 64..127 (ACT, in place)
        nc.scalar.square(S1[CM:2 * CM, :], S1[CM:2 * CM, :])
        nc.scalar.square(S2[CM:2 * CM, :], S2[CM:2 * CM, :])

        # ------------------------------------------------------------------
        # the big DMA streams, split over two DGE queues so descriptor
        # generation overlaps transfers:
        #  - l3 on the sync HARDWARE queue, f32
        #  - x2 on the gpsimd SOFTWARE queue: plain write of channels 0:128,
        #    then an accumulate-max DMA of channels 128:256 onto the same
        #    tile, halving the DVE reduce work
        # ------------------------------------------------------------------
        sp = ctx.enter_context(tc.tile_pool(name="sp", bufs=3))
        l3t = []
        x2t = []

        def issue_l3(bp):
            t = sp.tile([P, TB, 8, 128], F32, name="l3t", tag="l3t")
            nc.sync.dma_start(
                t[:], l3d[TB * bp: TB * (bp + 1)].rearrange("b (q j) c -> q b j c", j=8)
            )
            l3t.append(t)

        def issue_x2(bp):
            t = sp.tile([P, TB, 8, 256], BF16, name="x2t", tag="x2t")
            nc.gpsimd.dma_start(
                t[:], x2d[TB * bp: TB * (bp + 1)].rearrange("b (q j) c -> q b j c", j=8)
            )
            x2t.append(t)

        def red_l3(bp):
            nc.vector.tensor_reduce(
                xt[:, TB * bp: TB * (bp + 1), :], l3t[bp][:], axis=AX.X, op=ALU.max
            )

        def red_x2(bp):
            # bf16 tensor_tensor max over channel halves (2x-pipe eligible),
            # then the segmented reduce over the remaining 128
            t = x2t[bp]
            m = sp.tile([P, TB, 8, 128], BF16, name="x2m", tag="x2m")
            nc.vector.tensor_tensor(
                m[:], t[:, :, :, 0:128], t[:, :, :, 128:256], ALU.max
            )
            nc.vector.tensor_reduce(
                xx[:, TB * bp: TB * (bp + 1), :], m[:], axis=AX.X, op=ALU.max
            )

        # issue every stream DMA up front; pool semaphores pace them
        for bp in range(NT):
            issue_l3(bp)
        for bp in range(NT):
            issue_x2(bp)

        # ------------------------------------------------------------------
        # memory-addressing branches, phase-batched
        # ------------------------------------------------------------------
        bctx = ExitStack()
        bpsum = bctx.enter_context(tc.tile_pool(name="bpsum", bufs=2, space="PSUM"))
        brs = bctx.enter_context(tc.tile_pool(name="brs", bufs=2))

        lss = []
        for bi, S in enumerate([S1, S2]):
            lp = bpsum.tile([P, NCHUNK * 17], F32, name=f"lss{bi}", tag="lss")
            for c in range(NCHUNK):
                nc.tensor.matmul(
                    lp[:, c * 17: (c + 1) * 17],
                    lhsT=S[:, c * P: (c + 1) * P],
                    rhs=W("rhs2"),
                    start=True,
                    stop=True,
                )
            lss.append(lp)
        for bi in range(2):
            nc.scalar.activation(
                rr[:, bi * 16: bi * 16 + 16],
                lss[bi].rearrange("p (c k) -> p c k", k=17)[:, :, 16],
                AF.Sqrt,
                bias=eps[:],
            )

        red_l3(0)
        red_l3(1)
        red_x2(0)
        nc.vector.reciprocal(rinv[:], rr[:])

        for bi in range(2):
            for c in range(NCHUNK):
                nc.scalar.activation(
                    epk[bi][:, c, :],
                    lss[bi][:, c * 17: c * 17 + 16],
                    AF.Exp,
                    scale=rinv[:, bi * 16 + c: bi * 16 + c + 1],
                )
        red_l3(2)
        red_l3(3)
        red_x2(1)
        for bi in range(2):
            nc.vector.tensor_reduce(
                se[:, bi * 16: bi * 16 + 16], epk[bi][:], axis=AX.X, op=ALU.add
            )
        nc.vector.reciprocal(rs[:], se[:])
        for bi in range(2):
            for c in range(NCHUNK):
                nc.scalar.activation(
                    apk[bi][:, c, :],
                    epk[bi][:, c, :],
                    AF.Copy,
                    scale=rs[:, bi * 16 + c: bi * 16 + c + 1],
                )

        red_l3(4)
        red_l3(5)
        red_x2(2)

        # per-group mlp pipelines: branch 1 then branch 2
        def branch_group(bi, g, M1, M2, xm):
            aTp = bpsum.tile([16, 512], BF16, name="aTp", tag="aTp")
            for chn in range(4):
                nc.tensor.transpose(
                    aTp[:, chn * P: (chn + 1) * P],
                    apk[bi][:, g * 4 + chn, :],
                    W("identb"),
                )
            aT = brs.tile([16, 512], BF16, name="aT", tag="aT")
            nc.scalar.copy(aT[:], aTp[:])
            y1 = brs.tile([P, M1 // P, 512], BF16, name=f"y1_{bi}", tag=f"y1_{bi}")
            for mj in range(M1 // P):
                y1p = bpsum.tile([P, 512], F32, name="y1p", tag="y1p")
                nc.tensor.matmul(
                    y1p[:],
                    lhsT=W(f"w1eT_b{bi + 1}")[0:16, mj * P: (mj + 1) * P],
                    rhs=aT[:],
                    start=True,
                    stop=True,
                )
                nc.scalar.activation(
                    y1[:, mj, :], y1p[:], AF.Relu,
                    bias=Wf(f"b1_b{bi + 1}")[:, mj: mj + 1],
                )
            for mj2 in range(M2 // P):
                y2p = bpsum.tile([P, 512], F32, name="y2p", tag="y2p")
                for kc in range(M1 // P):
                    nc.tensor.matmul(
                        y2p[:],
                        lhsT=W(f"w2T_b{bi + 1}")[:, kc * M2 + mj2 * P: kc * M2 + (mj2 + 1) * P],
                        rhs=y1[:, kc, :],
                        start=(kc == 0),
                        stop=(kc == M1 // P - 1),
                    )
                y2 = brs.tile([P, 512], BF16, name="y2", tag="y2")
                nc.scalar.activation(
                    y2[:], y2p[:], AF.Relu,
                    bias=Wf(f"b2_b{bi + 1}")[:, mj2: mj2 + 1],
                )
                nc.vector.tensor_reduce(
                    xm[:, mj2, g * 16: (g + 1) * 16],
                    y2.rearrange("p (b n) -> p b n", n=NM),
                    axis=AX.X,
                    op=ALU.max,
                )

        for g in range(NGROUP):
            branch_group(0, g, 128, 256, xm1)
        red_l3(6)
        red_l3(7)
        red_x2(3)
        red_l3(8)
        red_l3(9)
        red_x2(4)
        for g in range(NGROUP):
            branch_group(1, g, 256, 512, xm2)
        red_l3(10)
        red_l3(11)
        red_x2(5)
        red_l3(12)
        red_l3(13)
        red_x2(6)
        red_l3(14)
        red_l3(15)
        bctx.close()

        # ------------------------------------------------------------------
        # heads
        # ------------------------------------------------------------------
        hpsum = ctx.enter_context(tc.tile_pool(name="hpsum", bufs=1, space="PSUM"))
        hs = ctx.enter_context(tc.tile_pool(name="hs", bufs=1))

        def logsoftmax(hi, f3, odram):
            zp = hpsum.tile([BL, 40], F32, name=f"zp{hi}", tag=f"zp{hi}")
            nc.tensor.transpose(zp[:], f3[:], Wf("identf")[0:40, 0:40])
            z = hs.tile([BL, 40], F32, name=f"z_{hi}", tag=f"z{hi}")
            nc.scalar.copy(z[:], zp[:])
            nm = hs.tile([BL, 1], F32, name=f"hnm{hi}", tag=f"hnm{hi}")
            nc.vector.tensor_reduce(nm[:], z[:], axis=AX.X, op=ALU.max, negate=True)
            e = hs.tile([BL, 40], F32, name=f"he{hi}", tag=f"he{hi}")
            sse = hs.tile([BL, 1], F32, name=f"hse{hi}", tag=f"hse{hi}")
            nc.scalar.activation(e[:], z[:], AF.Exp, bias=nm[:], accum_out=sse[:])
            lse = hs.tile([BL, 1], F32, name=f"lse{hi}", tag=f"lse{hi}")
            nc.scalar.activation(lse[:], sse[:], AF.Ln)
            oo = hs.tile([BL, 40], F32, name=f"oo_{hi}", tag=f"oo{hi}")
            nc.vector.tensor_scalar(oo[:], z[:], nm[:], lse[:], ALU.add, ALU.subtract)
            nc.sync.dma_start(odram, oo[:])

        # ---- head 1: everything available once l3 stream + branch1 done
        rhs1 = [xm1[:, j, :] for j in range(2)] + [xt[:, :, j] for j in range(8)]
        pp1 = hpsum.tile([P, 4, BL], F32, name="pp1", tag="pp1")
        h1a = hs.tile([P, 4, BL], BF16, name="h1a", tag="h1a")
        for mj in range(4):
            for kc in range(10):
                nc.tensor.matmul(
                    pp1[:, mj, :],
                    lhsT=W("fw1_h1")[:, kc * 512 + mj * P: kc * 512 + (mj + 1) * P],
                    rhs=rhs1[kc],
                    start=(kc == 0),
                    stop=(kc == 9),
                )
            nc.scalar.activation(
                h1a[:, mj, :], pp1[:, mj, :], AF.Relu, bias=Wf("fb1_h1")[:, mj: mj + 1]
            )
        pp2 = hpsum.tile([P, 2, BL], F32, name="pp2", tag="pp2")
        h2a = hs.tile([P, 2, BL], BF16, name="h2a", tag="h2a")
        for mj in range(2):
            for kc in range(4):
                nc.tensor.matmul(
                    pp2[:, mj, :],
                    lhsT=W("fw2_h1")[:, kc * 256 + mj * P: kc * 256 + (mj + 1) * P],
                    rhs=h1a[:, kc, :],
                    start=(kc == 0),
                    stop=(kc == 3),
                )
            nc.scalar.activation(
                h2a[:, mj, :], pp2[:, mj, :], AF.Relu, bias=Wf("fb2_h1")[:, mj: mj + 1]
            )
        pp3 = hpsum.tile([40, BL], F32, name="pp3", tag="pp3")
        for kc in range(2):
            nc.tensor.matmul(
                pp3[:],
                lhsT=W("fw3_h1")[:, kc * 40: (kc + 1) * 40],
                rhs=h2a[:, kc, :],
                start=(kc == 0),
                stop=(kc == 1),
            )
        f31 = hs.tile([40, BL], F32, name="f31", tag="f31")
        nc.scalar.activation(f31[:], pp3[:], AF.Identity, bias=Wf("fb3_h1")[0:40, 0:1])
        logsoftmax(0, f31, o1d)

        # ---- head 2: mem+l3 K-chunks accumulate now into qm; x2 chunks
        # accumulate per batch-half into qx; combine qm+qx before the relu.
        rhs2m = [xm2[:, j, :] for j in range(4)] + [xt[:, :, j] for j in range(8)]
        qm = hpsum.tile([P, 4, BL], F32, name="qm", tag="qm")
        qx = hpsum.tile([P, 4, BL], F32, name="qx", tag="qx")
        qmS = hs.tile([P, 4, BL], F32, name="qmS", tag="qmS")
        hsum = hs.tile([P, 4, BL], F32, name="hsum", tag="hsum")
        q1a = hs.tile([P, 4, BL], BF16, name="q1a", tag="h1a")
        for mj in range(4):
            for kc in range(12):
                nc.tensor.matmul(
                    qm[:, mj, :],
                    lhsT=W("fw1_h2")[:, kc * 512 + mj * P: kc * 512 + (mj + 1) * P],
                    rhs=rhs2m[kc],
                    start=(kc == 0),
                    stop=(kc == 11),
                )
            nc.scalar.copy(qmS[:, mj, :], qm[:, mj, :])

        red_x2(7)

        qp2 = hpsum.tile([P, 2, BL], F32, name="qp2", tag="pp2")
        q2a = hs.tile([P, 2, BL], BF16, name="q2a", tag="h2a")
        qp3 = hpsum.tile([40, BL], F32, name="qp3", tag="pp3")
        f32_ = hs.tile([40, BL], F32, name="f32_", tag="f31")

        def head2_half(h):
            bs = slice(h * 32, (h + 1) * 32)
            for mj in range(4):
                for j in range(8):
                    nc.tensor.matmul(
                        qx[:, mj, bs],
                        lhsT=W("fw1_h2")[:, (4 + j) * 512 + mj * P: (4 + j) * 512 + (mj + 1) * P],
                        rhs=xx[:, bs, j],
                        start=(j == 0),
                        stop=(j == 7),
                    )
                nc.vector.tensor_tensor(
                    hsum[:, mj, bs], qmS[:, mj, bs], qx[:, mj, bs], ALU.add
                )
                nc.scalar.activation(
                    q1a[:, mj, bs], hsum[:, mj, bs], AF.Relu,
                    bias=Wf("fb1_h2")[:, mj: mj + 1],
                )
            for mj in range(2):
                for kc in range(4):
                    nc.tensor.matmul(
                        qp2[:, mj, bs],
                        lhsT=W("fw2_h2")[:, kc * 256 + mj * P: kc * 256 + (mj + 1) * P],
                        rhs=q1a[:, kc, bs],
                        start=(kc == 0),
                        stop=(kc == 3),
                    )
                nc.scalar.activation(
                    q2a[:, mj, bs], qp2[:, mj, bs], AF.Relu,
                    bias=Wf("fb2_h2")[:, mj: mj + 1],
                )
            for kc in range(2):
                nc.tensor.matmul(
                    qp3[:, bs],
                    lhsT=W("fw3_h2")[:, kc * 40: (kc + 1) * 40],
                    rhs=q2a[:, kc, bs],
                    start=(kc == 0),
                    stop=(kc == 1),
                )
            nc.scalar.activation(
                f32_[:, bs], qp3[:, bs], AF.Identity, bias=Wf("fb3_h2")[0:40, 0:1]
            )

        # first half of x2 batches is complete after tile 7
        head2_half(0)
        red_x2(8)
        red_x2(9)
        red_x2(10)
        red_x2(11)
        red_x2(12)
        red_x2(13)
        red_x2(14)
        red_x2(15)
        head2_half(1)
        logsoftmax(1, f32_, o2d)

        if os.environ.get("KERNEL_DEBUG"):
            for nm_, t, w in [
                ("dxt", xt, BL * 8), ("dxx", xx, BL * 8),
                ("dxm1", xm1, 2 * BL), ("dxm2", xm2, 4 * BL),
            ]:
                dt_ = nc.dram_tensor(nm_, [P, w], BF16, kind="ExternalOutput").ap()
                nc.sync.dma_start(dt_, t[:].rearrange("p a b -> p (a b)"))
            for nm_, t in [("df1", f31), ("df2", f32_)]:
                dt_ = nc.dram_tensor(nm_, [40, BL], F32, kind="ExternalOutput").ap()
                nc.sync.dma_start(dt_, t[:])

    nc.compile()
    return nc


# ----------------------------------------------------------------------------
# entry point
# ----------------------------------------------------------------------------
_CACHE = {}


def _prep(inputs):
    f = {k: np.ascontiguousarray(np.asarray(v), dtype=np.float32) for k, v in inputs.items()}
    wpb, offb, wpf, offf = _fold_and_pack(f)
    if "nc" not in _CACHE:
        _CACHE["nc"] = _build(offb, wpb.shape[1], offf, wpf.shape[1])
    in_maps = []
    for c in range(NCORES):
        sl = slice(c * BL, (c + 1) * BL)
        in_maps.append(
            {
                "l3": np.ascontiguousarray(f["l3_points"][sl]),
                "x2": np.ascontiguousarray(f["x2_points"][sl]),
                "mf1": np.ascontiguousarray(
                    np.transpose(f["mem_f1"][sl], (1, 0, 2)).reshape(CM, ROWS).astype(NPBF)
                ),
                "mf2": np.ascontiguousarray(
                    np.transpose(f["mem_f2"][sl], (1, 0, 2)).reshape(CM, ROWS).astype(NPBF)
                ),
                "wpb": wpb,
                "wpf": wpf,
            }
        )
    return _CACHE["nc"], in_maps


def _run(inputs, trace=False):
    nc, in_maps = _prep(inputs)
    res = run_bass_kernel_spmd(nc, in_maps, core_ids=list(range(NCORES)), trace=trace)
    out1 = np.concatenate([res.results[c]["out1"] for c in range(NCORES)], axis=0)
    out2 = np.concatenate([res.results[c]["out2"] for c in range(NCORES)], axis=0)
    return (out1, out2), res


def kernel(**inputs):
    (out1, out2), _ = _run(inputs, trace=bool(os.environ.get("KERNEL_TRACE")))
    return out1, out2


# revision 21
# speedup vs baseline: 1.0403x; 1.0403x over previous
# Bass/Trainium2 kernel for nn_MENet (scatter_memory).
#
# Strategy: pure data parallel over batch (512 -> 64 per core, 8 cores).
#
# The kernel is HBM-bound: each core must stream 101.7MB of l3/x2 points
# (floor ~280us at ~360GB/s). Everything else is organized to hide under
# that stream:
#   - l3/x2 tiles are DMAd via gpsimd software-DGE with an in-flight
#     f32->bf16 cast; the gpsimd queue carries ONLY these 32 DMAs, issued
#     up-front, so the stream starts at t=0 and never stalls on compute.
#   - DVE does the segmented row maxes in bf16 (2x pipe) plus two tiny
#     reciprocals; all other pointwise work lives on ACT/PE.
#   - memory-addressing softmax: logits are q-hat . m-hat / ||q|| with
#     |logit|<=1, so no max subtraction; exp(l * rinv) is one ACT op per
#     128-row chunk using the per-partition scale operand. ACT functions
#     are phase-batched (square, sqrt, exp, scale, relu) to avoid
#     activation-table reloads.
#   - all matmul weights/activations are bf16 (PE 1 cycle/row vs 4 for
#     f32); psum accumulation stays f32.
#   - head2 consumes max(l3) and max(x2) as separate K-chunks of fc1_2
#     (W(u+v) = Wu + Wv), removing the elementwise adds; its x2-dependent
#     matmuls run per batch-half so only ~32 batches of work trail the
#     last x2 tile.
import os
from contextlib import ExitStack

import numpy as np
import ml_dtypes

import concourse.bacc as bacc
import concourse.bass as bass
import concourse.tile as tile
from concourse import mybir
from concourse.bass_utils import run_bass_kernel_spmd

F32 = mybir.dt.float32
BF16 = mybir.dt.bfloat16
NPBF = ml_dtypes.bfloat16
AF = mybir.ActivationFunctionType
ALU = mybir.AluOpType
AX = mybir.AxisListType

P = 128
NCORES = 8
B = 512
BL = B // NCORES          # 64 batches per core
NM = 32                   # n points per memory block
CM = 64                   # memory channel dim
ROWS = BL * NM            # 2048 rows per core per branch
NCHUNK = ROWS // P        # 16 chunks of 128 rows
NGROUP = ROWS // 512      # 4 groups of 512 rows (16 batches each)
TB = 4                    # batches per streamed tile
NT = BL // TB             # 16 tiles per stream
EPS_BN = 1e-5


# ----------------------------------------------------------------------------
# host-side weight folding + packing (bf16 matmul pack + f32 bias pack)
# ----------------------------------------------------------------------------
class _Pack:
    def __init__(self, dtype):
        self.dtype = dtype
        self.parts = []
        self.off = {}
        self.pos = 0

    def add(self, name, arr):
        arr = np.asarray(arr, np.float32)
        assert arr.ndim == 2 and arr.shape[0] <= P
        buf = np.zeros((P, arr.shape[1]), np.float32)
        buf[: arr.shape[0]] = arr
        self.off[name] = (self.pos, arr.shape[1])
        self.pos += arr.shape[1]
        self.parts.append(buf)

    def finish(self):
        return np.ascontiguousarray(
            np.concatenate(self.parts, axis=1).astype(self.dtype)
        )


def _kpack(w_t):  # [K, M] -> [128, nk, M] flattened to [128, nk*M]
    K, M = w_t.shape
    nk = K // P
    return np.ascontiguousarray(
        np.transpose(w_t.reshape(nk, P, M), (1, 0, 2)).reshape(P, nk * M)
    )


def _perm_pts(npref, npts):
    # device x-vector position npref + j*128 + q  <-  original point 8q + j
    d = np.arange(npts)
    src = npref + 8 * (d % 128) + (d // 128)
    return np.concatenate([np.arange(npref), src])


def _fold_and_pack(f):
    s = lambda g: g / np.sqrt(1.0 + EPS_BN)
    mw = f["memory_w"]                                    # [16, 64]
    mn = mw / np.maximum(np.linalg.norm(mw, axis=1, keepdims=True), 1e-12)

    pb = _Pack(NPBF)       # bf16: matmul operands
    pf = _Pack(np.float32)  # f32: biases, f32 identity

    pb.add("identb", np.eye(P, dtype=np.float32))
    rhs2 = np.zeros((P, 17), np.float32)
    rhs2[0:CM, 0:16] = mn.T                               # logits part
    rhs2[CM:2 * CM, 16] = 1.0                             # sum-of-squares part
    pb.add("rhs2", rhs2)

    for bi, (w1, g1, b1, w2, g2, b2) in enumerate(
        [
            (f["mlp1_w1"], f["mlp1_g1"], f["mlp1_b1"], f["mlp1_w2"], f["mlp1_g2"], f["mlp1_b2"]),
            (f["mlp2_w1"], f["mlp2_g1"], f["mlp2_b1"], f["mlp2_w2"], f["mlp2_g2"], f["mlp2_b2"]),
        ]
    ):
        w1e = (s(g1)[:, None] * w1) @ mw.T                # [M1, 16]
        w2f = s(g2)[:, None] * w2                         # [M2, M1]
        M1, M2 = w2f.shape[1], w2f.shape[0]
        pb.add(f"w1eT_b{bi + 1}", w1e.T)                  # [16, M1]
        pf.add(f"b1_b{bi + 1}", b1.reshape(M1 // P, P).T)
        pb.add(f"w2T_b{bi + 1}", _kpack(w2f.T))           # [128, (M1/128)*M2]
        pf.add(f"b2_b{bi + 1}", b2.reshape(M2 // P, P).T)

    for hi, (w1, b1, g1, bb1, w2, b2, g2, bb2, w3, b3, npref) in enumerate(
        [
            (f["fc1_w"], f["fc1_b"], f["bn1_g"], f["bn1_b"], f["fc2_w"], f["fc2_b"],
             f["bn2_g"], f["bn2_b"], f["fc3_w"], f["fc3_b"], 256),
            (f["fc1_2_w"], f["fc1_2_b"], f["bn1_2_g"], f["bn1_2_b"], f["fc2_2_w"],
             f["fc2_2_b"], f["bn2_2_g"], f["bn2_2_b"], f["fc3_2_w"], f["fc3_2_b"], 512),
        ]
    ):
        s1, s2 = s(g1), s(g2)
        w1f = (s1[:, None] * w1)[:, _perm_pts(npref, 1024)]   # [512, npref+1024]
        b1f = s1 * b1 + bb1
        w2f = s2[:, None] * w2                                # [256, 512]
        b2f = s2 * b2 + bb2
        pb.add(f"fw1_h{hi + 1}", _kpack(w1f.T))               # [128, nk1*512]
        pf.add(f"fb1_h{hi + 1}", b1f.reshape(4, P).T)
        pb.add(f"fw2_h{hi + 1}", _kpack(w2f.T))               # [128, 4*256]
        pf.add(f"fb2_h{hi + 1}", b2f.reshape(2, P).T)
        pb.add(f"fw3_h{hi + 1}", _kpack(w3.T))                # [128, 2*40]
        pf.add(f"fb3_h{hi + 1}", b3.reshape(40, 1))

    pf.add("identf", np.eye(40, dtype=np.float32))

    return pb.finish(), pb.off, pf.finish(), pf.off


# ----------------------------------------------------------------------------
# device program
# ----------------------------------------------------------------------------
def _build(offb, NWB, offf, NWF):
    nc = bacc.Bacc("TRN2", target_bir_lowering=False, debug=False)
    l3d = nc.dram_tensor("l3", [BL, 1024, 128], F32, kind="ExternalInput").ap()
    x2d = nc.dram_tensor("x2", [BL, 1024, 256], F32, kind="ExternalInput").ap()
    mf1d = nc.dram_tensor("mf1", [CM, ROWS], BF16, kind="ExternalInput").ap()
    mf2d = nc.dram_tensor("mf2", [CM, ROWS], BF16, kind="ExternalInput").ap()
    wbd = nc.dram_tensor("wpb", [P, NWB], BF16, kind="ExternalInput").ap()
    wfd = nc.dram_tensor("wpf", [P, NWF], F32, kind="ExternalInput").ap()
    o1d = nc.dram_tensor("out1", [BL, 40], F32, kind="ExternalOutput").ap()
    o2d = nc.dram_tensor("out2", [BL, 40], F32, kind="ExternalOutput").ap()

    # branch-weight prefix of the bf16 pack (needed within ~5us; the big
    # head-weight tail can land later)
    bw_end = offb["fw1_h1"][0]

    with tile.TileContext(nc) as tc, ExitStack() as ctx:
        pp = ctx.enter_context(tc.tile_pool(name="persist", bufs=1))
        wb = pp.tile([P, NWB], BF16, name="wb")
        wf = pp.tile([P, NWF], F32, name="wf")
        S1 = pp.tile([P, ROWS], BF16, name="S1")
        S2 = pp.tile([P, ROWS], BF16, name="S2")
        # scalar (ACT) hardware queue: memory-branch inputs + weights, so the
        # sync queue is free for the l3 stream from t=0
        nc.scalar.dma_start(S1[0:CM, :], mf1d)
        nc.scalar.dma_start(S1[CM:2 * CM, :], mf1d)
        nc.scalar.dma_start(S2[0:CM, :], mf2d)
        nc.scalar.dma_start(S2[CM:2 * CM, :], mf2d)
        nc.scalar.dma_start(wb[:, 0:bw_end], wbd[:, 0:bw_end])
        nc.scalar.dma_start(wf[:], wfd)
        nc.scalar.dma_start(wb[:, bw_end:], wbd[:, bw_end:])

        def W(name):
            o, w = offb[name]
            return wb[:, o: o + w]

        def Wf(name):
            o, w = offf[name]
            return wf[:, o: o + w]

        eps = pp.tile([P, 1], F32, name="eps")
        nc.vector.memset(eps[:], 1e-24)

        xt = pp.tile([P, BL, 8], BF16, name="xt")     # l3 point maxes
        xx = pp.tile([P, BL, 8], BF16, name="xx")     # x2 point maxes
        xm1 = pp.tile([P, 2, BL], BF16, name="xm1")   # branch1 mlp maxes
        xm2 = pp.tile([P, 4, BL], BF16, name="xm2")   # branch2 mlp maxes
        rr = pp.tile([P, 32], F32, name="rr")         # ||q|| per chunk-row
        rinv = pp.tile([P, 32], F32, name="rinv")
        se = pp.tile([P, 32], F32, name="se")
        rs = pp.tile([P, 32], F32, name="rs")
        epk = [pp.tile([P, NCHUNK, 16], BF16, name=f"e{b}") for b in range(2)]
        apk = [pp.tile([P, NCHUNK, 16], BF16, name=f"a{b}") for b in range(2)]

        # squares on partitions 64..127 (ACT, in place)
        nc.scalar.square(S1[CM:2 * CM, :], S1[CM:2 * CM, :])
        nc.scalar.square(S2[CM:2 * CM, :], S2[CM:2 * CM, :])

        # ------------------------------------------------------------------
        # the big DMA streams, split over two DGE queues so descriptor
        # generation overlaps transfers:
        #  - l3 on the sync HARDWARE queue, f32
        #  - x2 on the gpsimd SOFTWARE queue: plain write of channels 0:128,
        #    then an accumulate-max DMA of channels 128:256 onto the same
        #    tile, halving the DVE reduce work
        # ------------------------------------------------------------------
        sp = ctx.enter_context(tc.tile_pool(name="sp", bufs=3))
        l3t = []
        x2t = []

        def issue_l3(bp):
            t = sp.tile([P, TB, 8, 128], F32, name="l3t", tag="l3t")
            eng = nc.sync if bp % 2 == 0 else nc.scalar
            eng.dma_start(
                t[:], l3d[TB * bp: TB * (bp + 1)].rearrange("b (q j) c -> q b j c", j=8)
            )
            l3t.append(t)

        def issue_x2(bp):
            t = sp.tile([P, TB, 8, 256], BF16, name="x2t", tag="x2t")
            nc.gpsimd.dma_start(
                t[:], x2d[TB * bp: TB * (bp + 1)].rearrange("b (q j) c -> q b j c", j=8)
            )
            x2t.append(t)

        def red_l3(bp):
            nc.vector.tensor_reduce(
                xt[:, TB * bp: TB * (bp + 1), :], l3t[bp][:], axis=AX.X, op=ALU.max
            )

        def red_x2(bp):
            # bf16 tensor_tensor max over channel halves (2x-pipe eligible),
            # then the segmented reduce over the remaining 128
            t = x2t[bp]
            m = sp.tile([P, TB, 8, 128], BF16, name="x2m", tag="x2m")
            nc.vector.tensor_tensor(
                m[:], t[:, :, :, 0:128], t[:, :, :, 128:256], ALU.max
            )
            nc.vector.tensor_reduce(
                xx[:, TB * bp: TB * (bp + 1), :], m[:], axis=AX.X, op=ALU.max
            )

        # issue every stream DMA up front; pool semaphores pace them
        for bp in range(NT):
            issue_l3(bp)
        for bp in range(NT):
            issue_x2(bp)

        # ------------------------------------------------------------------
        # memory-addressing branches, phase-batched
        # ------------------------------------------------------------------
        bctx = ExitStack()
        bpsum = bctx.enter_context(tc.tile_pool(name="bpsum", bufs=2, space="PSUM"))
        brs = bctx.enter_context(tc.tile_pool(name="brs", bufs=2))

        lss = []
        for bi, S in enumerate([S1, S2]):
            lp = bpsum.tile([P, NCHUNK * 17], F32, name=f"lss{bi}", tag="lss")
            for c in range(NCHUNK):
                nc.tensor.matmul(
                    lp[:, c * 17: (c + 1) * 17],
                    lhsT=S[:, c * P: (c + 1) * P],
                    rhs=W("rhs2"),
                    start=True,
                    stop=True,
                )
            lss.append(lp)
        for bi in range(2):
            nc.scalar.activation(
                rr[:, bi * 16: bi * 16 + 16],
                lss[bi].rearrange("p (c k) -> p c k", k=17)[:, :, 16],
                AF.Sqrt,
                bias=eps[:],
            )

        red_l3(0)
        red_l3(1)
        red_x2(0)
        nc.vector.reciprocal(rinv[:], rr[:])

        for bi in range(2):
            for c in range(NCHUNK):
                nc.scalar.activation(
                    epk[bi][:, c, :],
                    lss[bi][:, c * 17: c * 17 + 16],
                    AF.Exp,
                    scale=rinv[:, bi * 16 + c: bi * 16 + c + 1],
                )
        red_l3(2)
        red_l3(3)
        red_x2(1)
        for bi in range(2):
            nc.vector.tensor_reduce(
                se[:, bi * 16: bi * 16 + 16], epk[bi][:], axis=AX.X, op=ALU.add
            )
        nc.vector.reciprocal(rs[:], se[:])
        for bi in range(2):
            for c in range(NCHUNK):
                nc.scalar.activation(
                    apk[bi][:, c, :],
                    epk[bi][:, c, :],
                    AF.Copy,
                    scale=rs[:, bi * 16 + c: bi * 16 + c + 1],
                )

        red_l3(4)
        red_l3(5)
        red_x2(2)

        # per-group mlp pipelines: branch 1 then branch 2
        def branch_group(bi, g, M1, M2, xm):
            aTp = bpsum.tile([16, 512], BF16, name="aTp", tag="aTp")
            for chn in range(4):
                nc.tensor.transpose(
                    aTp[:, chn * P: (chn + 1) * P],
                    apk[bi][:, g * 4 + chn, :],
                    W("identb"),
                )
            aT = brs.tile([16, 512], BF16, name="aT", tag="aT")
            nc.scalar.copy(aT[:], aTp[:])
            y1 = brs.tile([P, M1 // P, 512], BF16, name=f"y1_{bi}", tag=f"y1_{bi}")
            for mj in range(M1 // P):
                y1p = bpsum.tile([P, 512], F32, name="y1p", tag="y1p")
                nc.tensor.matmul(
                    y1p[:],
                    lhsT=W(f"w1eT_b{bi + 1}")[0:16, mj * P: (mj + 1) * P],
                    rhs=aT[:],
                    start=True,
                    stop=True,
                )
                nc.scalar.activation(
                    y1[:, mj, :], y1p[:], AF.Relu,
                    bias=Wf(f"b1_b{bi + 1}")[:, mj: mj + 1],
                )
            for mj2 in range(M2 // P):
                y2p = bpsum.tile([P, 512], F32, name="y2p", tag="y2p")
                for kc in range(M1 // P):
                    nc.tensor.matmul(
                        y2p[:],
                        lhsT=W(f"w2T_b{bi + 1}")[:, kc * M2 + mj2 * P: kc * M2 + (mj2 + 1) * P],
                        rhs=y1[:, kc, :],
                        start=(kc == 0),
                        stop=(kc == M1 // P - 1),
                    )
                y2 = brs.tile([P, 512], BF16, name="y2", tag="y2")
                nc.scalar.activation(
                    y2[:], y2p[:], AF.Relu,
                    bias=Wf(f"b2_b{bi + 1}")[:, mj2: mj2 + 1],
                )
                nc.vector.tensor_reduce(
                    xm[:, mj2, g * 16: (g + 1) * 16],
                    y2.rearrange("p (b n) -> p b n", n=NM),
                    axis=AX.X,
                    op=ALU.max,
                )

        for g in range(NGROUP):
            branch_group(0, g, 128, 256, xm1)
        red_l3(6)
        red_l3(7)
        red_x2(3)
        red_l3(8)
        red_l3(9)
        red_x2(4)
        for g in range(NGROUP):
            branch_group(1, g, 256, 512, xm2)
        red_l3(10)
        red_l3(11)
        red_x2(5)
        red_l3(12)
        red_l3(13)
        red_x2(6)
        red_l3(14)
        red_l3(15)
        bctx.close()

        # ------------------------------------------------------------------
        # heads
        # ------------------------------------------------------------------
        hpsum = ctx.enter_context(tc.tile_pool(name="hpsum", bufs=1, space="PSUM"))
        hs = ctx.enter_context(tc.tile_pool(name="hs", bufs=1))

        def logsoftmax(hi, f3, odram):
            zp = hpsum.tile([BL, 40], F32, name=f"zp{hi}", tag=f"zp{hi}")
            nc.tensor.transpose(zp[:], f3[:], Wf("identf")[0:40, 0:40])
            z = hs.tile([BL, 40], F32, name=f"z_{hi}", tag=f"z{hi}")
            nc.scalar.copy(z[:], zp[:])
            nm = hs.tile([BL, 1], F32, name=f"hnm{hi}", tag=f"hnm{hi}")
            nc.vector.tensor_reduce(nm[:], z[:], axis=AX.X, op=ALU.max, negate=True)
            e = hs.tile([BL, 40], F32, name=f"he{hi}", tag=f"he{hi}")
            sse = hs.tile([BL, 1], F32, name=f"hse{hi}", tag=f"hse{hi}")
            nc.scalar.activation(e[:], z[:], AF.Exp, bias=nm[:], accum_out=sse[:])
            lse = hs.tile([BL, 1], F32, name=f"lse{hi}", tag=f"lse{hi}")
            nc.scalar.activation(lse[:], sse[:], AF.Ln)
            oo = hs.tile([BL, 40], F32, name=f"oo_{hi}", tag=f"oo{hi}")
            nc.vector.tensor_scalar(oo[:], z[:], nm[:], lse[:], ALU.add, ALU.subtract)
            nc.sync.dma_start(odram, oo[:])

        # ---- head 1: everything available once l3 stream + branch1 done
        rhs1 = [xm1[:, j, :] for j in range(2)] + [xt[:, :, j] for j in range(8)]
        pp1 = hpsum.tile([P, 4, BL], F32, name="pp1", tag="pp1")
        h1a = hs.tile([P, 4, BL], BF16, name="h1a", tag="h1a")
        for mj in range(4):
            for kc in range(10):
                nc.tensor.matmul(
                    pp1[:, mj, :],
                    lhsT=W("fw1_h1")[:, kc * 512 + mj * P: kc * 512 + (mj + 1) * P],
                    rhs=rhs1[kc],
                    start=(kc == 0),
                    stop=(kc == 9),
                )
            nc.scalar.activation(
                h1a[:, mj, :], pp1[:, mj, :], AF.Relu, bias=Wf("fb1_h1")[:, mj: mj + 1]
            )
        pp2 = hpsum.tile([P, 2, BL], F32, name="pp2", tag="pp2")
        h2a = hs.tile([P, 2, BL], BF16, name="h2a", tag="h2a")
        for mj in range(2):
            for kc in range(4):
                nc.tensor.matmul(
                    pp2[:, mj, :],
                    lhsT=W("fw2_h1")[:, kc * 256 + mj * P: kc * 256 + (mj + 1) * P],
                    rhs=h1a[:, kc, :],
                    start=(kc == 0),
                    stop=(kc == 3),
                )
            nc.scalar.activation(
                h2a[:, mj, :], pp2[:, mj, :], AF.Relu, bias=Wf("fb2_h1")[:, mj: mj + 1]
            )
        pp3 = hpsum.tile([40, BL], F32, name="pp3", tag="pp3")
        for kc in range(2):
            nc.tensor.matmul(
                pp3[:],
                lhsT=W("fw3_h1")[:, kc * 40: (kc + 1) * 40],
                rhs=h2a[:, kc, :],
                start=(kc == 0),
                stop=(kc == 1),
            )
        f31 = hs.tile([40, BL], F32, name="f31", tag="f31")
        nc.scalar.activation(f31[:], pp3[:], AF.Identity, bias=Wf("fb3_h1")[0:40, 0:1])
        logsoftmax(0, f31, o1d)

        # ---- head 2: mem+l3 K-chunks accumulate now into qm; x2 chunks
        # accumulate per batch-half into qx; combine qm+qx before the relu.
        rhs2m = [xm2[:, j, :] for j in range(4)] + [xt[:, :, j] for j in range(8)]
        qm = hpsum.tile([P, 4, BL], F32, name="qm", tag="qm")
        qx = hpsum.tile([P, 4, BL], F32, name="qx", tag="qx")
        qmS = hs.tile([P, 4, BL], F32, name="qmS", tag="qmS")
        hsum = hs.tile([P, 4, BL], F32, name="hsum", tag="hsum")
        q1a = hs.tile([P, 4, BL], BF16, name="q1a", tag="h1a")
        for mj in range(4):
            for kc in range(12):
                nc.tensor.matmul(
                    qm[:, mj, :],
                    lhsT=W("fw1_h2")[:, kc * 512 + mj * P: kc * 512 + (mj + 1) * P],
                    rhs=rhs2m[kc],
                    start=(kc == 0),
                    stop=(kc == 11),
                )
            nc.scalar.copy(qmS[:, mj, :], qm[:, mj, :])

        red_x2(7)

        qp2 = hpsum.tile([P, 2, BL], F32, name="qp2", tag="pp2")
        q2a = hs.tile([P, 2, BL], BF16, name="q2a", tag="h2a")
        qp3 = hpsum.tile([40, BL], F32, name="qp3", tag="pp3")
        f32_ = hs.tile([40, BL], F32, name="f32_", tag="f31")

        def head2_half(h):
            bs = slice(h * 32, (h + 1) * 32)
            for mj in range(4):
                for j in range(8):
                    nc.tensor.matmul(
                        qx[:, mj, bs],
                        lhsT=W("fw1_h2")[:, (4 + j) * 512 + mj * P: (4 + j) * 512 + (mj + 1) * P],
                        rhs=xx[:, bs, j],
                        start=(j == 0),
                        stop=(j == 7),
                    )
                nc.vector.tensor_tensor(
                    hsum[:, mj, bs], qmS[:, mj, bs], qx[:, mj, bs], ALU.add
                )
                nc.scalar.activation(
                    q1a[:, mj, bs], hsum[:, mj, bs], AF.Relu,
                    bias=Wf("fb1_h2")[:, mj: mj + 1],
                )
            for mj in range(2):
                for kc in range(4):
                    nc.tensor.matmul(
                        qp2[:, mj, bs],
                        lhsT=W("fw2_h2")[:, kc * 256 + mj * P: kc * 256 + (mj + 1) * P],
                        rhs=q1a[:, kc, bs],
                        start=(kc == 0),
                        stop=(kc == 3),
                    )
                nc.scalar.activation(
                    q2a[:, mj, bs], qp2[:, mj, bs], AF.Relu,
                    bias=Wf("fb2_h2")[:, mj: mj + 1],
                )
            for kc in range(2):
                nc.tensor.matmul(
                    qp3[:, bs],
                    lhsT=W("fw3_h2")[:, kc * 40: (kc + 1) * 40],
                    rhs=q2a[:, kc, bs],
                    start=(kc == 0),
                    stop=(kc == 1),
                )
            nc.scalar.activation(
                f32_[:, bs], qp3[:, bs], AF.Identity, bias=Wf("fb3_h2")[0:40, 0:1]
            )

        # first half of x2 batches is complete after tile 7
        head2_half(0)
        red_x2(8)
        red_x2(9)
        red_x2(10)
        red_x2(11)
        red_x2(12)
        red_x2(13)
        red_x2(14)
        red_x2(15)
        head2_half(1)
        logsoftmax(1, f32_, o2d)

        if os.environ.get("KERNEL_DEBUG"):
            for nm_, t, w in [
                ("dxt", xt, BL * 8), ("dxx", xx, BL * 8),
                ("dxm1", xm1, 2 * BL), ("dxm2", xm2, 4 * BL),
            ]:
                dt_ = nc.dram_tensor(nm_, [P, w], BF16, kind="ExternalOutput").ap()
                nc.sync.dma_start(dt_, t[:].rearrange("p a b -> p (a b)"))
            for nm_, t in [("df1", f31), ("df2", f32_)]:
                dt_ = nc.dram_tensor(nm_, [40, BL], F32, kind="ExternalOutput").ap()
                nc.sync.dma_start(dt_, t[:])

    nc.compile()
    return nc


# ----------------------------------------------------------------------------
# entry point
# ----------------------------------------------------------------------------
_CACHE = {}


def _prep(inputs):
    f = {k: np.ascontiguousarray(np.asarray(v), dtype=np.float32) for k, v in inputs.items()}
    wpb, offb, wpf, offf = _fold_and_pack(f)
    if "nc" not in _CACHE:
        _CACHE["nc"] = _build(offb, wpb.shape[1], offf, wpf.shape[1])
    in_maps = []
    for c in range(NCORES):
        sl = slice(c * BL, (c + 1) * BL)
        in_maps.append(
            {
                "l3": np.ascontiguousarray(f["l3_points"][sl]),
                "x2": np.ascontiguousarray(f["x2_points"][sl]),
                "mf1": np.ascontiguousarray(
                    np.transpose(f["mem_f1"][sl], (1, 0, 2)).reshape(CM, ROWS).astype(NPBF)
                ),
                "mf2": np.ascontiguousarray(
                    np.transpose(f["mem_f2"][sl], (1, 0, 2)).reshape(CM, ROWS).astype(NPBF)
                ),
                "wpb": wpb,
                "wpf": wpf,
            }
        )
    return _CACHE["nc"], in_maps


def _run(inputs, trace=False):
    nc, in_maps = _prep(inputs)
    res = run_bass_kernel_spmd(nc, in_maps, core_ids=list(range(NCORES)), trace=trace)
    out1 = np.concatenate([res.results[c]["out1"] for c in range(NCORES)], axis=0)
    out2 = np.concatenate([res.results[c]["out2"] for c in range(NCORES)], axis=0)
    return (out1, out2), res


def kernel(**inputs):
    (out1, out2), _ = _run(inputs, trace=bool(os.environ.get("KERNEL_TRACE")))
    return out1, out2
